# revision 32
# baseline (speedup 1.0000x reference)
"""GIN (3-layer) message-passing kernel for 8 Trainium2 NeuronCores.

Strategy (spmd, one program image for all 8 cores, 2 device launches):
  - 1D node partition: core c owns dst nodes [c*N/8, (c+1)*N/8).
  - Algebraic refactors:
      * layer(h) = relu((h + A@h) @ W + b): the gather feeds on the RAW
        node table h (not h@W), so layer 1 needs no separate dense
        launch -- launch A gathers straight from the x table.
      * out = segment_sum(h3, batch) = [P^T (I+A) h2] @ W3 + counts b3^T
        with P = onehot(batch). M := (I + A^T) P is host-computable from
        the edge list, so layer 3 + global pool collapse into a tiny
        per-window matmul at the end of launch B. No third launch.
  - Everything fp16 on device (PE 1 cyc/row vs 4 for fp32), PSUM f32.
  - Aggregation in transposed form: for each 128-edge tile,
      preT[feat, slot] += gathered^T @ S   (lhsT = gathered tile,
      rhs = S[e, slot] = (iota==slot_e) * w_e built by one DVE op)
    so no per-window transposes are needed: preT is directly the lhsT
    of the dense W matmul (h_win = (preT_win)^T @ W + b).
  - Edge groups (window, quadrant) are packed back-to-back at slot
    granularity (size = max-over-cores count, no 128 rounding). A tile
    spanning g groups runs g full-128 matmuls, one per group, each with
    its own weight column that is zero outside the group's span. Gather
    descriptors therefore carry only the core-imbalance padding (~10%)
    instead of ~25%.
  - Stream order: super-batches of SB*WB windows, quadrant-major inside,
    so dma_gather calls fill the 1024-index ucode cap (fewer calls ->
    less fixed SWDGE descriptor-gen time on Pool).
  - The "+h" self term enters via one full-bank identity matmul per
    batch; bias via one K=1 matmul per batch (exactly one PSUM
    accumulation group per bank: start on the first matmul, stop on the
    last -- opening a second group in a bank discards the first).
  - Launch A: x-table gathers -> h1 = relu((x + A@x)@W1 + b1) rows.
    Host glues h1 (concat core rows) into the launch-B table.
    Launch B: h1-table gathers -> h2 rows -> q_c = M_c^T @ h2_c [G,HID].
    Host: out = (sum_c q_c) @ W3 + counts b3^T.
"""

import numpy as np
import concourse.bass as bass
import concourse.mybir as mybir
import concourse.tile as tile
from concourse import bacc
from concourse.bass_utils import run_bass_kernel_spmd

F32 = mybir.dt.float32
F16 = mybir.dt.float16
I16 = mybir.dt.int16
AOT = mybir.AluOpType
ACT = mybir.ActivationFunctionType

NCORES = 8
WIN = 128           # dst rows per psum window
WB = 4              # windows per batch (one 512-col psum bank)
SB = 4              # batches per super-batch (gather-call scope)
CALL_TILES = 8      # 128-edge tiles per dma_gather call (1024 idx ucode cap)
SCRATCH = 16384     # dynamic dma scratch -> 1024-descriptor SWDGE ring


class Cfg:
    def __init__(self, N, E, IN=128, HID=128, C=40, G=64):
        assert N % (4 * NCORES) == 0
        self.N, self.E, self.IN, self.HID, self.C, self.G = N, E, IN, HID, C, G
        self.NPC = N // NCORES            # nodes per core
        self.NW = -(-self.NPC // WIN)     # windows per core
        self.NPAD = self.NW * WIN
        self.QROWS = N // 4               # nodes per quadrant (int16 idx cap)


class Plan:
    """Edge partition shared by both launches. Structure (tile counts /
    call layout / per-tile op lists) is identical across cores; only the
    per-core data arrays differ. Group sizes are the max-over-cores edge
    count; per-core shortfall slots gather row 0 with weight 0."""

    def __init__(self, cfg, src, dst, ew, c_of, w_of, slot_of, qa, srow):
        self.cfg = cfg
        NPC, NW, QR = cfg.NPC, cfg.NW, cfg.QROWS
        core = c_of[dst]
        w = w_of[dst]
        slot = slot_of[dst]
        q = qa[src]                       # balanced quadrant assignment
        srcl = srow[src] % QR

        cnt = np.zeros((NCORES, NW, 4), np.int64)
        np.add.at(cnt, (core, w, q), 1)
        gsz = cnt.max(axis=0)                          # [NW, 4] group slots

        # layout: super-batches of SB*WB windows; inside, quadrant-major
        # runs; groups packed back-to-back at slot granularity.
        self.supers = []     # (batch list, c_lo, c_hi, t_lo, t_hi)
        self.calls = []      # (q, t0, ntiles, nvalid)
        self.tile_ops = []   # per tile: list of (op_idx, window)
        group_base = np.zeros((NW, 4), np.int64)       # slot offset of group
        n_ops = 0
        op_arr = {}          # (window, quadrant) -> first op idx of group
        op_tile0 = {}        # (window, quadrant) -> first tile of group
        trail_spans = []     # structural trailing pad -> idx -1
        t_cursor = 0
        for s0 in range(0, NW, SB * WB):
            s1 = min(s0 + SB * WB, NW)
            batches = [(b0, min(b0 + WB, s1)) for b0 in range(s0, s1, WB)]
            c_lo = len(self.calls)
            t_lo = t_cursor
            for qq in range(4):
                run_t0 = t_cursor
                s_cursor = run_t0 * 128                # slot cursor
                for ww in range(s0, s1):
                    group_base[ww, qq] = s_cursor
                    s_cursor += gsz[ww, qq]
                run_tiles = -(-(s_cursor - run_t0 * 128) // 128)
                t_cursor = run_t0 + max(run_tiles, 0)
                # per-tile op list for this run
                for t in range(run_t0, t_cursor):
                    lo, hi = t * 128, (t + 1) * 128
                    tops = []
                    for ww in range(s0, s1):
                        gb = group_base[ww, qq]
                        if max(lo, gb) < min(hi, gb + gsz[ww, qq]):
                            if (ww, qq) not in op_arr:
                                op_arr[(ww, qq)] = n_ops
                                op_tile0[(ww, qq)] = t
                            tops.append((n_ops, ww))
                            n_ops += 1
                    assert tops, "tile with no group coverage"
                    self.tile_ops.append(tops)
                # chunk this run into gather calls; run-end padding slots
                # become trailing -1 idxs of the run's last call
                pad = t_cursor * 128 - s_cursor
                if pad:
                    trail_spans.append((s_cursor, t_cursor * 128))
                t = run_t0
                while t < t_cursor:
                    n = min(CALL_TILES, t_cursor - t)
                    nvalid = n * 128
                    if t + n == t_cursor:
                        nvalid -= pad
                    self.calls.append((qq, t, n, nvalid))
                    t += n
            self.supers.append((batches, c_lo, len(self.calls), t_lo, t_cursor))
        self.NT = t_cursor
        self.NOPS = n_ops
        assert len(self.tile_ops) == self.NT

        # last tile containing ops of each batch (for tail emission) and
        # last op of each batch (psum stop flag)
        self.batch_last_tile = {}
        self.batch_last_op = {}
        for t, tops in enumerate(self.tile_ops):
            for op, ww in tops:
                b = ww // WB
                self.batch_last_tile[b] = t
                self.batch_last_op[b] = op

        # per-core padded data arrays (edges by group rank)
        order = np.lexsort((q, w, core))           # edge order by (core,w,q)
        g_of_edge = (core * NW + w) * 4 + q
        gb_flat = group_base.reshape(-1)           # [NW*4] slot offsets
        sorted_g = g_of_edge[order]
        starts = np.searchsorted(sorted_g, np.arange(NCORES * NW * 4))
        rank = np.arange(len(order)) - starts[sorted_g]
        pos = gb_flat[(w * 4 + q)] + rank[np.argsort(order, kind="stable")]
        # pos: slot position of each edge in its core's padded stream
        # op idx of an edge = group's first op + (edge tile - group's first
        # tile); ops of one group occupy consecutive indices in tile order
        tile_of = pos // 128
        part_of = pos % 128
        op0 = np.zeros((NW, 4), np.int64)
        t0g = np.zeros((NW, 4), np.int64)
        for (ww, qq), o in op_arr.items():
            op0[ww, qq] = o
            t0g[ww, qq] = op_tile0[(ww, qq)]
        opidx = op0[w, q] + (tile_of - t0g[w, q])
        self.idx = np.zeros((NCORES, self.NT * 128), np.int16)
        self.slot = np.zeros((NCORES, self.NT * 128), np.float32)
        self.wgt = np.zeros((NCORES, self.NOPS * 128), np.float32)
        self.idx[core, pos] = srcl.astype(np.int16)
        self.slot[core, pos] = slot.astype(np.float32)
        self.wgt[core, opidx * 128 + part_of] = ew.astype(np.float32)
        for s_lo, s_hi in trail_spans:
            self.idx[:, s_lo:s_hi] = -1

    def idx_wrapped(self, c):
        # idx j -> partition j%16, col j//16; replicated to 128 partitions
        a = self.idx[c].reshape(-1, 16).T          # [16, NT*8]
        return np.ascontiguousarray(np.tile(a, (8, 1)))

    def col_arr(self, a, c, nt):
        # [nt*128] -> [128, nt] (partition = position in tile)
        return np.ascontiguousarray(a[c].reshape(nt, 128).T)


def _iota_tile(n, m):
    return np.tile(np.arange(m, dtype=np.float16), (n, 1))


def balance_maps(cfg, src, dst):
    """Host-side load balancing. Returns
      c_of/w_of/slot_of: dst node -> (core, window, slot)  (serpentine by
        in-degree so every (core, window) has near-equal total degree)
      qa/srow: src node -> quadrant / table row  (greedy: flatten the
        per-(core, window, quadrant) edge counts the SPMD padding is
        sized by)"""
    N, NW, QR = cfg.N, cfg.NW, cfg.QROWS
    nbins = NCORES * NW
    # --- dst: serpentine deal by in-degree ---
    deg_in = np.bincount(dst, minlength=N)
    order = np.argsort(-deg_in, kind="stable")
    i = np.arange(N)
    rnd, p = i // nbins, i % nbins
    binp = np.where(rnd % 2 == 0, p, nbins - 1 - p)
    c_of = np.empty(N, np.int64)
    w_of = np.empty(N, np.int64)
    slot_of = np.empty(N, np.int64)
    c_of[order] = binp // NW
    w_of[order] = binp % NW
    slot_of[order] = rnd
    # --- src: chunked greedy quadrant assignment ---
    cw = c_of[dst] * NW + w_of[dst]                # [E] window bin of edge
    ordE = np.argsort(src, kind="stable")
    src_s, cw_s = src[ordE], cw[ordE]
    ptr = np.searchsorted(src_s, np.arange(N + 1))
    deg_out = ptr[1:] - ptr[:-1]
    norder = np.argsort(-deg_out, kind="stable")
    cnt = np.zeros((nbins * NW * 0 + nbins, 4), np.float64)
    cntm = np.zeros((nbins, 4), np.float64)
    fill = np.zeros(4, np.int64)
    qa = np.full(N, -1, np.int8)
    CAP = QR
    CH = 256
    for lo in range(0, N, CH):
        nodes = norder[lo:lo + CH]
        segs = [np.arange(ptr[n], ptr[n + 1]) for n in nodes]
        lens = np.array([len(s) for s in segs])
        if lens.sum() == 0:
            sc = np.zeros((len(nodes), 4))
        else:
            eidx = np.concatenate(segs)
            own = np.repeat(np.arange(len(nodes)), lens)
            sc = np.zeros((len(nodes), 4))
            np.add.at(sc, own, cntm[cw_s[eidx]])
        sc = sc + np.where(fill >= CAP, np.inf, 0.0)
        qsel = np.argmin(sc, axis=1)
        # capacity-aware: spill overflow picks until stable
        for _ in range(8):
            over = False
            for qq in range(4):
                picks = np.where(qsel == qq)[0]
                room = CAP - fill[qq]
                if len(picks) > room:
                    over = True
                    sc[picks[room:], qq] = np.inf
                    qsel[picks[room:]] = np.argmin(sc[picks[room:]], axis=1)
            if not over:
                break
        for qq in range(4):
            fill[qq] += int((qsel == qq).sum())
        qa[nodes] = qsel
        if lens.sum():
            np.add.at(cntm, (cw_s[eidx], qsel[own]), 1.0)
    assert (qa >= 0).all() and (np.bincount(qa, minlength=4) <= CAP).all()
    # table rows: rank within quadrant
    qorder = np.argsort(qa, kind="stable")
    rank = np.empty(N, np.int64)
    qs = qa[qorder]
    qstart = np.searchsorted(qs, np.arange(4))
    rank[qorder] = np.arange(N) - qstart[qs]
    srow = qa.astype(np.int64) * QR + rank
    # node_at[c, w, slot] = node id (-1 for pad slots)
    node_at = np.full((NCORES, NW, 128), -1, np.int64)
    node_at[c_of, w_of, slot_of] = np.arange(N)
    return c_of, w_of, slot_of, qa, srow, node_at


def build_layer(cfg, plan, pool):
    """One launch:
         preT[:, win] = hT_own[win] + (A@h)^T[win]        (psum, f32)
         h_next[win] = relu(preT[:, win]^T @ W + b)       (fp16)
         launch A (pool=False): h_next rows -> DRAM
         launch B (pool=True):  q += M_win^T @ h_next_win (psum, f32)
    """
    nc = bacc.Bacc("TRN2", target_bir_lowering=False, debug=False,
                   num_devices=NCORES, dynamic_dma_scratch_size=SCRATCH)
    ht_d = nc.dram_tensor("ht", [cfg.N, cfg.HID], F16, kind="ExternalInput").ap()
    hTw_d = nc.dram_tensor("hTw", [128, cfg.NW * 128], F16,
                           kind="ExternalInput").ap()
    id_d = nc.dram_tensor("ident", [128, 128], F16, kind="ExternalInput").ap()
    io_d = nc.dram_tensor("iota", [128, 128], F16, kind="ExternalInput").ap()
    ix_d = nc.dram_tensor("eidx", [128, plan.NT * 8], I16, kind="ExternalInput").ap()
    sl_d = nc.dram_tensor("eslot", [128, plan.NT], F32, kind="ExternalInput").ap()
    wg_d = nc.dram_tensor("ewgt", [128, plan.NOPS], F32, kind="ExternalInput").ap()
    w_d = nc.dram_tensor("W", [cfg.HID, cfg.HID], F16, kind="ExternalInput").ap()
    b_d = nc.dram_tensor("brow", [1, WB * cfg.HID], F16, kind="ExternalInput").ap()
    on_d = nc.dram_tensor("ones1", [1, 128], F16, kind="ExternalInput").ap()
    if pool:
        m_d = nc.dram_tensor("M", [128, cfg.NW * cfg.G], F16,
                             kind="ExternalInput").ap()
        out_d = nc.dram_tensor("q", [cfg.G, cfg.HID], F32,
                               kind="ExternalOutput").ap()
    else:
        out_d = nc.dram_tensor("h_out", [cfg.NPAD, cfg.HID], F16,
                               kind="ExternalOutput").ap()
        out_r = out_d.rearrange("(n p) d -> p n d", p=128)
    uq = [ht_d[i * cfg.QROWS:(i + 1) * cfg.QROWS, :] for i in range(4)]
    hTw_r = hTw_d.rearrange("p (n d) -> p n d", d=128)
    m_r = m_d.rearrange("p (n g) -> p n g", g=cfg.G) if pool else None

    with tile.TileContext(nc) as tc:
        with tc.tile_pool(name="const", bufs=1) as cst, \
             tc.tile_pool(name="hw", bufs=2 * SB + 1) as hw, \
             tc.tile_pool(name="gath", bufs=14) as gath, \
             tc.tile_pool(name="sp", bufs=12) as sp, \
             tc.tile_pool(name="io", bufs=3) as io, \
             tc.tile_pool(name="aps", bufs=SB + 1, space="PSUM") as aps, \
             tc.tile_pool(name="hps", bufs=2, space="PSUM") as hps, \
             tc.tile_pool(name="ops", bufs=1, space="PSUM") as ops:
            id_sb = cst.tile([128, 128], F16)
            nc.sync.dma_start(out=id_sb[:], in_=id_d[:])
            iota_sb = cst.tile([128, 128], F16)
            nc.sync.dma_start(out=iota_sb[:], in_=io_d[:])
            w_sb = cst.tile([cfg.HID, cfg.HID], F16)
            nc.sync.dma_start(out=w_sb[:], in_=w_d[:])
            b_sb = cst.tile([1, WB * cfg.HID], F16)
            nc.sync.dma_start(out=b_sb[:], in_=b_d[:])
            on_sb = cst.tile([1, 128], F16)
            nc.sync.dma_start(out=on_sb[:], in_=on_d[:])
            ixall = cst.tile([128, plan.NT * 8], I16)
            nc.sync.dma_start(out=ixall[:], in_=ix_d[:])
            slall = cst.tile([128, plan.NT], F32)
            nc.sync.dma_start(out=slall[:], in_=sl_d[:])
            wgall = cst.tile([128, plan.NOPS], F32)
            nc.sync.dma_start(out=wgall[:], in_=wg_d[:])
            if pool:
                q_ps = ops.tile([cfg.G, cfg.HID], F32, tag="q")

            for (batches, c_lo, c_hi, t_lo, t_hi) in plan.supers:
                bbase = batches[0][0] // WB
                # own-rows hT window chunks (and M chunks) for the super
                hT_t, m_t, pre_ps = {}, {}, {}
                for bi, (b0, b1) in enumerate(batches):
                    nb = b1 - b0
                    hT_t[bi] = hw.tile([128, nb, 128], F16, tag="hT", name=f"hT_{bi}")
                    nc.sync.dma_start(out=hT_t[bi][:], in_=hTw_r[:, b0:b1, :])
                    if pool:
                        m_t[bi] = hw.tile([128, nb, cfg.G], F16, tag="m", name=f"m_{bi}")
                        nc.sync.dma_start(out=m_t[bi][:], in_=m_r[:, b0:b1, :])
                # gather calls for the super
                tile_src = {}      # tile idx -> (sbuf tile, col)
                for ci in range(c_lo, c_hi):
                    qq, t0, ntl, nvalid = plan.calls[ci]
                    gt = gath.tile([128, ntl, cfg.HID], F16, tag="gt")
                    nc.gpsimd.dma_gather(
                        gt[:], uq[qq], ixall[:, t0 * 8:(t0 + ntl) * 8],
                        ntl * 128, nvalid, cfg.HID)
                    for k in range(ntl):
                        tile_src[t0 + k] = (gt, k)
                # aggregation matmuls; exactly one accumulation group per
                # psum bank (start on the identity matmul, stop on the
                # batch's last op)
                for t in range(t_lo, t_hi):
                    for op, ww in plan.tile_ops[t]:
                        bi = ww // WB - bbase
                        b0, b1 = batches[bi]
                        if bi not in pre_ps:
                            pre_ps[bi] = aps.tile([128, b1 - b0, 128], F32,
                                                  tag="pre", name=f"pre_{bi}")
                            nc.tensor.matmul(out=pre_ps[bi][:], lhsT=id_sb[:],
                                             rhs=hT_t[bi][:],
                                             start=True, stop=False)
                        s_t = sp.tile([128, 128], F16, tag="S")
                        nc.vector.tensor_scalar(
                            out=s_t[:], in0=iota_sb[:],
                            scalar1=slall[:, t:t + 1],
                            scalar2=wgall[:, op:op + 1],
                            op0=AOT.is_equal, op1=AOT.mult)
                        gt, k = tile_src[t]
                        nc.tensor.matmul(
                            out=pre_ps[bi][:, ww - b0, :],
                            lhsT=gt[:, k, :], rhs=s_t[:],
                            start=False,
                            stop=(op == plan.batch_last_op[ww // WB]))
                    # dense tail for any batch whose ops just finished
                    for bi, (b0, b1) in enumerate(batches):
                        if plan.batch_last_tile.get(b0 // WB) != t:
                            continue
                        nb = b1 - b0
                        preT_sb = io.tile([128, nb, 128], F16, tag="preT")
                        nc.scalar.activation(out=preT_sb[:], in_=pre_ps[bi][:],
                                             func=ACT.Copy)
                        h_ps = hps.tile([128, nb, 128], F32, tag="h")
                        nc.tensor.matmul(out=h_ps[:], lhsT=on_sb[:],
                                         rhs=b_sb[:, :nb * 128],
                                         start=True, stop=False)
                        for wb in range(nb):
                            nc.tensor.matmul(out=h_ps[:, wb, :],
                                             lhsT=preT_sb[:, wb, :],
                                             rhs=w_sb[:], start=False,
                                             stop=(wb == nb - 1))
                        h_sb = io.tile([128, nb, 128], F16, tag="hsb")
                        nc.scalar.activation(out=h_sb[:], in_=h_ps[:],
                                             func=ACT.Relu)
                        if pool:
                            for wb in range(nb):
                                nc.tensor.matmul(
                                    out=q_ps[:], lhsT=m_t[bi][:, wb, :],
                                    rhs=h_sb[:, wb, :],
                                    start=(b0 + wb == 0),
                                    stop=(b0 + wb == cfg.NW - 1))
                        else:
                            nc.sync.dma_start(out=out_r[:, b0:b1, :],
                                              in_=h_sb[:])
            if pool:
                q_sb = io.tile([cfg.G, cfg.HID], F32, tag="qsb")
                nc.vector.tensor_copy(out=q_sb[:], in_=q_ps[:])
                nc.sync.dma_start(out=out_d[:], in_=q_sb[:])
    nc.compile()
    return nc


RUNNER = None      # test harness hook: replaces the device-run path


def _run(nc, in_maps):
    if RUNNER is not None:
        return RUNNER(nc, in_maps)
    return run_bass_kernel_spmd(nc, in_maps, core_ids=list(range(NCORES))).results


def _win_major(a, cfg):
    """[NPC, D] row table -> [128, NW, D] (partition = slot in window)."""
    d = a.shape[1]
    out = np.zeros((cfg.NPAD, d), a.dtype)
    out[:a.shape[0]] = a
    return np.ascontiguousarray(
        out.reshape(cfg.NW, 128, d).transpose(1, 0, 2))


def _layer_inputs(cfg, plan, table_f16, bal, ident, iota, ones1, W, b):
    """table_f16: full [N, HID] fp16 node table (node-id order); bal is
    the balance_maps tuple -- the uploaded gather table is permuted to
    table row order and own rows follow node_at."""
    c_of, w_of, slot_of, qa, srow, node_at = bal
    inv = np.empty(cfg.N, np.int64)
    inv[srow] = np.arange(cfg.N)
    tab_perm = np.ascontiguousarray(table_f16[inv])
    maps = []
    for c in range(NCORES):
        flat = node_at[c].reshape(-1)
        mask = flat >= 0
        ownp = np.zeros((cfg.NPAD, cfg.HID), np.float16)
        ownp[mask] = table_f16[flat[mask]]
        # [128 feat, NW*128 nodes], window-major columns
        hTw = np.ascontiguousarray(
            ownp.reshape(cfg.NW, 128, cfg.HID).transpose(2, 0, 1)
            .reshape(cfg.HID, cfg.NW * 128))
        m = {"ht": tab_perm, "hTw": hTw, "ident": ident, "iota": iota,
             "ones1": ones1, "W": W,
             "brow": np.tile(b.reshape(1, -1), (1, WB)),
             "eidx": plan.idx_wrapped(c),
             "eslot": plan.col_arr(plan.slot, c, plan.NT),
             "ewgt": plan.col_arr(plan.wgt, c, plan.NOPS)}
        maps.append(m)
    return maps


NCS = {}


def gin_forward(cfg, x, edge_index, edge_weight, batch,
                W1, b1, W2, b2, W3, b3):
    src = np.asarray(edge_index[0], np.int64)
    dst = np.asarray(edge_index[1], np.int64)
    ew = np.asarray(edge_weight, np.float32)
    batch64 = np.asarray(batch, np.int64)
    bal = balance_maps(cfg, src, dst)
    c_of, w_of, slot_of, qa, srow, node_at = bal
    plan = Plan(cfg, src, dst, ew, c_of, w_of, slot_of, qa, srow)
    if "A" not in NCS:
        NCS["A"] = build_layer(cfg, plan, False)
        NCS["B"] = build_layer(cfg, plan, True)

    ident = np.eye(128, dtype=np.float16)
    iota = _iota_tile(128, 128)
    ones1 = np.ones((1, 128), np.float16)

    # M = (I + A^T) P  [N, G]: pool matrix, and per-graph node counts
    G = cfg.G
    M = np.bincount(src * G + batch64[dst], weights=ew.astype(np.float64),
                    minlength=cfg.N * G).reshape(cfg.N, G).astype(np.float32)
    M[np.arange(cfg.N), batch64] += 1.0
    counts = np.bincount(batch64, minlength=G).astype(np.float32)

    # Launch A: h1 = relu((x + A@x) @ W1 + b1)
    xt = np.ascontiguousarray(np.asarray(x, np.float32).astype(np.float16))
    maps = _layer_inputs(cfg, plan, xt, bal, ident, iota, ones1,
                         np.asarray(W1, np.float32).astype(np.float16),
                         np.asarray(b1, np.float32).astype(np.float16))
    res = _run(NCS["A"], maps)
    # h_out rows are in (core, window, slot) order; map back to node order
    h1_all = np.concatenate([res[c]["h_out"] for c in range(NCORES)])
    gidx = c_of * cfg.NPAD + w_of * 128 + slot_of
    h1 = np.ascontiguousarray(h1_all[gidx])

    # Launch B: h2 = relu((h1 + A@h1) @ W2 + b2); q_c = M_c^T @ h2_c
    maps = _layer_inputs(cfg, plan, h1, bal, ident, iota, ones1,
                         np.asarray(W2, np.float32).astype(np.float16),
                         np.asarray(b2, np.float32).astype(np.float16))
    M16 = M.astype(np.float16)
    for c, m in enumerate(maps):
        flat = node_at[c].reshape(-1)
        mask = flat >= 0
        mwin = np.zeros((cfg.NPAD, G), np.float16)
        mwin[mask] = M16[flat[mask]]
        m["M"] = np.ascontiguousarray(
            _win_major(mwin, cfg).reshape(128, cfg.NW * G))
    res = _run(NCS["B"], maps)
    q = np.zeros((G, cfg.HID), np.float32)
    for c in range(NCORES):
        q += res[c]["q"]

    out = q @ np.asarray(W3, np.float32) + \
        counts[:, None] * np.asarray(b3, np.float32)[None, :]
    return out.astype(np.float32)


def kernel(x, edge_index, edge_weight, batch, W1, b1, W2, b2, W3, b3):
    cfg = Cfg(N=100000, E=1600000)
    return gin_forward(cfg, x, edge_index, edge_weight, batch,
                       W1, b1, W2, b2, W3, b3)


# revision 34
# speedup vs baseline: 1.0142x; 1.0142x over previous
"""GIN (3-layer) message-passing kernel for 8 Trainium2 NeuronCores.

Strategy (spmd, one program image for all 8 cores, 2 device launches):
  - 1D node partition: core c owns dst nodes [c*N/8, (c+1)*N/8).
  - Algebraic refactors:
      * layer(h) = relu((h + A@h) @ W + b): the gather feeds on the RAW
        node table h (not h@W), so layer 1 needs no separate dense
        launch -- launch A gathers straight from the x table.
      * out = segment_sum(h3, batch) = [P^T (I+A) h2] @ W3 + counts b3^T
        with P = onehot(batch). M := (I + A^T) P is host-computable from
        the edge list, so layer 3 + global pool collapse into a tiny
        per-window matmul at the end of launch B. No third launch.
  - Everything fp16 on device (PE 1 cyc/row vs 4 for fp32), PSUM f32.
  - Aggregation in transposed form: for each 128-edge tile,
      preT[feat, slot] += gathered^T @ S   (lhsT = gathered tile,
      rhs = S[e, slot] = (iota==slot_e) * w_e built by one DVE op)
    so no per-window transposes are needed: preT is directly the lhsT
    of the dense W matmul (h_win = (preT_win)^T @ W + b).
  - Edge groups (window, quadrant) are packed back-to-back at slot
    granularity (size = max-over-cores count, no 128 rounding). A tile
    spanning g groups runs g full-128 matmuls, one per group, each with
    its own weight column that is zero outside the group's span. Gather
    descriptors therefore carry only the core-imbalance padding (~10%)
    instead of ~25%.
  - Stream order: super-batches of SB*WB windows, quadrant-major inside,
    so dma_gather calls fill the 1024-index ucode cap (fewer calls ->
    less fixed SWDGE descriptor-gen time on Pool).
  - The "+h" self term enters via one full-bank identity matmul per
    batch; bias via one K=1 matmul per batch (exactly one PSUM
    accumulation group per bank: start on the first matmul, stop on the
    last -- opening a second group in a bank discards the first).
  - Launch A: x-table gathers -> h1 = relu((x + A@x)@W1 + b1) rows.
    Host glues h1 (concat core rows) into the launch-B table.
    Launch B: h1-table gathers -> h2 rows -> q_c = M_c^T @ h2_c [G,HID].
    Host: out = (sum_c q_c) @ W3 + counts b3^T.
"""

import numpy as np
import concourse.bass as bass
import concourse.mybir as mybir
import concourse.tile as tile
from concourse import bacc
from concourse.bass_utils import run_bass_kernel_spmd

F32 = mybir.dt.float32
F16 = mybir.dt.float16
I16 = mybir.dt.int16
AOT = mybir.AluOpType
ACT = mybir.ActivationFunctionType

NCORES = 8
WIN = 128           # dst rows per psum window
WB = 4              # windows per batch (one 512-col psum bank)
SB = 4              # batches per super-batch (gather-call scope)
CALL_TILES = 8      # 128-edge tiles per dma_gather call (1024 idx ucode cap)
SCRATCH = 16384     # dynamic dma scratch -> 1024-descriptor SWDGE ring


class Cfg:
    def __init__(self, N, E, IN=128, HID=128, C=40, G=64):
        assert N % (4 * NCORES) == 0
        self.N, self.E, self.IN, self.HID, self.C, self.G = N, E, IN, HID, C, G
        self.NPC = N // NCORES            # nodes per core
        self.NW = -(-self.NPC // WIN)     # windows per core
        self.NPAD = self.NW * WIN
        self.QROWS = N // 4               # nodes per quadrant (int16 idx cap)


class Plan:
    """Edge partition shared by both launches. Structure (tile counts /
    call layout / per-tile op lists) is identical across cores; only the
    per-core data arrays differ. Group sizes are the max-over-cores edge
    count; per-core shortfall slots gather row 0 with weight 0."""

    def __init__(self, cfg, src, dst, ew, c_of, w_of, slot_of, qa, srow):
        self.cfg = cfg
        NPC, NW, QR = cfg.NPC, cfg.NW, cfg.QROWS
        core = c_of[dst]
        w = w_of[dst]
        slot = slot_of[dst]
        q = qa[src]                       # balanced quadrant assignment
        srcl = srow[src] % QR

        cnt = np.zeros((NCORES, NW, 4), np.int64)
        np.add.at(cnt, (core, w, q), 1)
        gsz = cnt.max(axis=0)                          # [NW, 4] group slots

        # layout: super-batches of SB*WB windows; inside, quadrant-major
        # runs; groups packed back-to-back at slot granularity.
        self.supers = []     # (batch list, c_lo, c_hi, t_lo, t_hi)
        self.calls = []      # (q, t0, ntiles, nvalid)
        self.tile_ops = []   # per tile: list of (op_idx, window)
        group_base = np.zeros((NW, 4), np.int64)       # slot offset of group
        n_ops = 0
        op_arr = {}          # (window, quadrant) -> first op idx of group
        op_tile0 = {}        # (window, quadrant) -> first tile of group
        trail_spans = []     # structural trailing pad -> idx -1
        t_cursor = 0
        for s0 in range(0, NW, SB * WB):
            s1 = min(s0 + SB * WB, NW)
            batches = [(b0, min(b0 + WB, s1)) for b0 in range(s0, s1, WB)]
            c_lo = len(self.calls)
            t_lo = t_cursor
            for qq in range(4):
                run_t0 = t_cursor
                s_cursor = run_t0 * 128                # slot cursor
                for ww in range(s0, s1):
                    group_base[ww, qq] = s_cursor
                    s_cursor += gsz[ww, qq]
                run_tiles = -(-(s_cursor - run_t0 * 128) // 128)
                t_cursor = run_t0 + max(run_tiles, 0)
                # per-tile op list for this run
                for t in range(run_t0, t_cursor):
                    lo, hi = t * 128, (t + 1) * 128
                    tops = []
                    for ww in range(s0, s1):
                        gb = group_base[ww, qq]
                        if max(lo, gb) < min(hi, gb + gsz[ww, qq]):
                            if (ww, qq) not in op_arr:
                                op_arr[(ww, qq)] = n_ops
                                op_tile0[(ww, qq)] = t
                            tops.append((n_ops, ww))
                            n_ops += 1
                    assert tops, "tile with no group coverage"
                    self.tile_ops.append(tops)
                # chunk this run into gather calls; run-end padding slots
                # become trailing -1 idxs of the run's last call
                pad = t_cursor * 128 - s_cursor
                if pad:
                    trail_spans.append((s_cursor, t_cursor * 128))
                t = run_t0
                while t < t_cursor:
                    n = min(CALL_TILES, t_cursor - t)
                    nvalid = n * 128
                    if t + n == t_cursor:
                        nvalid -= pad
                    self.calls.append((qq, t, n, nvalid))
                    t += n
            self.supers.append((batches, c_lo, len(self.calls), t_lo, t_cursor))
        self.NT = t_cursor
        self.NOPS = n_ops
        assert len(self.tile_ops) == self.NT

        # last tile containing ops of each batch (for tail emission) and
        # last op of each batch (psum stop flag)
        self.batch_last_tile = {}
        self.batch_last_op = {}
        for t, tops in enumerate(self.tile_ops):
            for op, ww in tops:
                b = ww // WB
                self.batch_last_tile[b] = t
                self.batch_last_op[b] = op

        # per-core padded data arrays (edges by group rank)
        order = np.lexsort((q, w, core))           # edge order by (core,w,q)
        g_of_edge = (core * NW + w) * 4 + q
        gb_flat = group_base.reshape(-1)           # [NW*4] slot offsets
        sorted_g = g_of_edge[order]
        starts = np.searchsorted(sorted_g, np.arange(NCORES * NW * 4))
        rank = np.arange(len(order)) - starts[sorted_g]
        pos = gb_flat[(w * 4 + q)] + rank[np.argsort(order, kind="stable")]
        # pos: slot position of each edge in its core's padded stream
        # op idx of an edge = group's first op + (edge tile - group's first
        # tile); ops of one group occupy consecutive indices in tile order
        tile_of = pos // 128
        part_of = pos % 128
        op0 = np.zeros((NW, 4), np.int64)
        t0g = np.zeros((NW, 4), np.int64)
        for (ww, qq), o in op_arr.items():
            op0[ww, qq] = o
            t0g[ww, qq] = op_tile0[(ww, qq)]
        opidx = op0[w, q] + (tile_of - t0g[w, q])
        self.idx = np.zeros((NCORES, self.NT * 128), np.int16)
        self.slot = np.zeros((NCORES, self.NT * 128), np.float32)
        self.wgt = np.zeros((NCORES, self.NOPS * 128), np.float32)
        self.idx[core, pos] = srcl.astype(np.int16)
        self.slot[core, pos] = slot.astype(np.float32)
        self.wgt[core, opidx * 128 + part_of] = ew.astype(np.float32)
        for s_lo, s_hi in trail_spans:
            self.idx[:, s_lo:s_hi] = -1

    def idx_wrapped(self, c):
        # idx j -> partition j%16, col j//16; replicated to 128 partitions
        a = self.idx[c].reshape(-1, 16).T          # [16, NT*8]
        return np.ascontiguousarray(np.tile(a, (8, 1)))

    def col_arr(self, a, c, nt):
        # [nt*128] -> [128, nt] (partition = position in tile)
        return np.ascontiguousarray(a[c].reshape(nt, 128).T)


def _iota_tile(n, m):
    return np.tile(np.arange(m, dtype=np.float16), (n, 1))


def balance_maps(cfg, src, dst):
    """Host-side load balancing. Returns
      c_of/w_of/slot_of: dst node -> (core, window, slot)  (serpentine by
        in-degree so every (core, window) has near-equal total degree)
      qa/srow: src node -> quadrant / table row  (greedy: flatten the
        per-(core, window, quadrant) edge counts the SPMD padding is
        sized by)"""
    N, NW, QR = cfg.N, cfg.NW, cfg.QROWS
    nbins = NCORES * NW
    # --- dst: serpentine deal by in-degree ---
    deg_in = np.bincount(dst, minlength=N)
    order = np.argsort(-deg_in, kind="stable")
    i = np.arange(N)
    rnd, p = i // nbins, i % nbins
    binp = np.where(rnd % 2 == 0, p, nbins - 1 - p)
    c_of = np.empty(N, np.int64)
    w_of = np.empty(N, np.int64)
    slot_of = np.empty(N, np.int64)
    c_of[order] = binp // NW
    w_of[order] = binp % NW
    slot_of[order] = rnd
    # --- src: chunked greedy quadrant assignment ---
    cw = c_of[dst] * NW + w_of[dst]                # [E] window bin of edge
    ordE = np.argsort(src, kind="stable")
    src_s, cw_s = src[ordE], cw[ordE]
    ptr = np.searchsorted(src_s, np.arange(N + 1))
    deg_out = ptr[1:] - ptr[:-1]
    norder = np.argsort(-deg_out, kind="stable")
    cnt = np.zeros((nbins * NW * 0 + nbins, 4), np.float64)
    cntm = np.zeros((nbins, 4), np.float64)
    fill = np.zeros(4, np.int64)
    qa = np.full(N, -1, np.int8)
    CAP = QR
    CH = 256
    for lo in range(0, N, CH):
        nodes = norder[lo:lo + CH]
        segs = [np.arange(ptr[n], ptr[n + 1]) for n in nodes]
        lens = np.array([len(s) for s in segs])
        if lens.sum() == 0:
            sc = np.zeros((len(nodes), 4))
        else:
            eidx = np.concatenate(segs)
            own = np.repeat(np.arange(len(nodes)), lens)
            sc = np.zeros((len(nodes), 4))
            np.add.at(sc, own, cntm[cw_s[eidx]])
        sc = sc + np.where(fill >= CAP, np.inf, 0.0)
        qsel = np.argmin(sc, axis=1)
        # capacity-aware: spill overflow picks until stable
        for _ in range(8):
            over = False
            for qq in range(4):
                picks = np.where(qsel == qq)[0]
                room = CAP - fill[qq]
                if len(picks) > room:
                    over = True
                    sc[picks[room:], qq] = np.inf
                    qsel[picks[room:]] = np.argmin(sc[picks[room:]], axis=1)
            if not over:
                break
        for qq in range(4):
            fill[qq] += int((qsel == qq).sum())
        qa[nodes] = qsel
        if lens.sum():
            np.add.at(cntm, (cw_s[eidx], qsel[own]), 1.0)
    assert (qa >= 0).all() and (np.bincount(qa, minlength=4) <= CAP).all()
    # table rows: rank within quadrant
    qorder = np.argsort(qa, kind="stable")
    rank = np.empty(N, np.int64)
    qs = qa[qorder]
    qstart = np.searchsorted(qs, np.arange(4))
    rank[qorder] = np.arange(N) - qstart[qs]
    srow = qa.astype(np.int64) * QR + rank
    # node_at[c, w, slot] = node id (-1 for pad slots)
    node_at = np.full((NCORES, NW, 128), -1, np.int64)
    node_at[c_of, w_of, slot_of] = np.arange(N)
    return c_of, w_of, slot_of, qa, srow, node_at


def build_layer(cfg, plan, pool):
    """One launch:
         preT[:, win] = hT_own[win] + (A@h)^T[win]        (psum, f32)
         h_next[win] = relu(preT[:, win]^T @ W + b)       (fp16)
         launch A (pool=False): h_next rows -> DRAM
         launch B (pool=True):  q += M_win^T @ h_next_win (psum, f32)
    """
    nc = bacc.Bacc("TRN2", target_bir_lowering=False, debug=False,
                   num_devices=NCORES, dynamic_dma_scratch_size=SCRATCH)
    ht_d = nc.dram_tensor("ht", [cfg.N, cfg.HID], F16, kind="ExternalInput").ap()
    hTw_d = nc.dram_tensor("hTw", [128, cfg.NW * 128], F16,
                           kind="ExternalInput").ap()
    id_d = nc.dram_tensor("ident", [128, 128], F16, kind="ExternalInput").ap()
    io_d = nc.dram_tensor("iota", [128, 128], F16, kind="ExternalInput").ap()
    ix_d = nc.dram_tensor("eidx", [128, plan.NT * 8], I16, kind="ExternalInput").ap()
    sl_d = nc.dram_tensor("eslot", [128, plan.NT], F32, kind="ExternalInput").ap()
    wg_d = nc.dram_tensor("ewgt", [128, plan.NOPS], F32, kind="ExternalInput").ap()
    w_d = nc.dram_tensor("W", [cfg.HID, cfg.HID], F16, kind="ExternalInput").ap()
    b_d = nc.dram_tensor("brow", [1, WB * cfg.HID], F16, kind="ExternalInput").ap()
    on_d = nc.dram_tensor("ones1", [1, 128], F16, kind="ExternalInput").ap()
    if pool:
        m_d = nc.dram_tensor("M", [128, cfg.NW * cfg.G], F16,
                             kind="ExternalInput").ap()
        out_d = nc.dram_tensor("q", [cfg.G, cfg.HID], F32,
                               kind="ExternalOutput").ap()
    else:
        out_d = nc.dram_tensor("h_out", [cfg.NPAD, cfg.HID], F16,
                               kind="ExternalOutput").ap()
        out_r = out_d.rearrange("(n p) d -> p n d", p=128)
    uq = [ht_d[i * cfg.QROWS:(i + 1) * cfg.QROWS, :] for i in range(4)]
    hTw_r = hTw_d.rearrange("p (n d) -> p n d", d=128)
    m_r = m_d.rearrange("p (n g) -> p n g", g=cfg.G) if pool else None

    with tile.TileContext(nc) as tc:
        with tc.tile_pool(name="const", bufs=1) as cst, \
             tc.tile_pool(name="hw", bufs=2 * SB + 1) as hw, \
             tc.tile_pool(name="gath", bufs=14) as gath, \
             tc.tile_pool(name="sp", bufs=12) as sp, \
             tc.tile_pool(name="io", bufs=3) as io, \
             tc.tile_pool(name="aps", bufs=SB + 1, space="PSUM") as aps, \
             tc.tile_pool(name="hps", bufs=2, space="PSUM") as hps, \
             tc.tile_pool(name="ops", bufs=1, space="PSUM") as ops:
            id_sb = cst.tile([128, 128], F16)
            nc.sync.dma_start(out=id_sb[:], in_=id_d[:])
            iota_sb = cst.tile([128, 128], F16)
            nc.sync.dma_start(out=iota_sb[:], in_=io_d[:])
            w_sb = cst.tile([cfg.HID, cfg.HID], F16)
            nc.sync.dma_start(out=w_sb[:], in_=w_d[:])
            b_sb = cst.tile([1, WB * cfg.HID], F16)
            nc.sync.dma_start(out=b_sb[:], in_=b_d[:])
            on_sb = cst.tile([1, 128], F16)
            nc.sync.dma_start(out=on_sb[:], in_=on_d[:])
            ixall = cst.tile([128, plan.NT * 8], I16)
            nc.sync.dma_start(out=ixall[:], in_=ix_d[:])
            slall = cst.tile([128, plan.NT], F32)
            nc.sync.dma_start(out=slall[:], in_=sl_d[:])
            wgall = cst.tile([128, plan.NOPS], F32)
            nc.sync.dma_start(out=wgall[:], in_=wg_d[:])
            if pool:
                q_ps = ops.tile([cfg.G, cfg.HID], F32, tag="q")

            for (batches, c_lo, c_hi, t_lo, t_hi) in plan.supers:
                bbase = batches[0][0] // WB
                # own-rows hT window chunks (and M chunks) for the super
                hT_t, m_t, pre_ps = {}, {}, {}
                for bi, (b0, b1) in enumerate(batches):
                    nb = b1 - b0
                    hT_t[bi] = hw.tile([128, nb, 128], F16, tag="hT", name=f"hT_{bi}")
                    nc.sync.dma_start(out=hT_t[bi][:], in_=hTw_r[:, b0:b1, :])
                    if pool:
                        m_t[bi] = hw.tile([128, nb, cfg.G], F16, tag="m", name=f"m_{bi}")
                        nc.sync.dma_start(out=m_t[bi][:], in_=m_r[:, b0:b1, :])
                # gather calls for the super
                tile_src = {}      # tile idx -> (sbuf tile, col)
                for ci in range(c_lo, c_hi):
                    qq, t0, ntl, nvalid = plan.calls[ci]
                    gt = gath.tile([128, ntl, cfg.HID], F16, tag="gt")
                    nc.gpsimd.dma_gather(
                        gt[:], uq[qq], ixall[:, t0 * 8:(t0 + ntl) * 8],
                        ntl * 128, nvalid, cfg.HID)
                    for k in range(ntl):
                        tile_src[t0 + k] = (gt, k)
                # aggregation matmuls; exactly one accumulation group per
                # psum bank (start on the identity matmul, stop on the
                # batch's last op)
                for t in range(t_lo, t_hi):
                    for op, ww in plan.tile_ops[t]:
                        bi = ww // WB - bbase
                        b0, b1 = batches[bi]
                        if bi not in pre_ps:
                            pre_ps[bi] = aps.tile([128, b1 - b0, 128], F32,
                                                  tag="pre", name=f"pre_{bi}")
                            nc.tensor.matmul(out=pre_ps[bi][:], lhsT=id_sb[:],
                                             rhs=hT_t[bi][:],
                                             start=True, stop=False)
                        s_t = sp.tile([128, 128], F16, tag="S")
                        nc.vector.tensor_scalar(
                            out=s_t[:], in0=iota_sb[:],
                            scalar1=slall[:, t:t + 1],
                            scalar2=wgall[:, op:op + 1],
                            op0=AOT.is_equal, op1=AOT.mult)
                        gt, k = tile_src[t]
                        nc.tensor.matmul(
                            out=pre_ps[bi][:, ww - b0, :],
                            lhsT=gt[:, k, :], rhs=s_t[:],
                            start=False,
                            stop=(op == plan.batch_last_op[ww // WB]))
                    # dense tail for any batch whose ops just finished
                    for bi, (b0, b1) in enumerate(batches):
                        if plan.batch_last_tile.get(b0 // WB) != t:
                            continue
                        nb = b1 - b0
                        preT_sb = io.tile([128, nb, 128], F16, tag="preT")
                        nc.scalar.activation(out=preT_sb[:], in_=pre_ps[bi][:],
                                             func=ACT.Copy)
                        h_ps = hps.tile([128, nb, 128], F32, tag="h")
                        nc.tensor.matmul(out=h_ps[:], lhsT=on_sb[:],
                                         rhs=b_sb[:, :nb * 128],
                                         start=True, stop=False)
                        for wb in range(nb):
                            nc.tensor.matmul(out=h_ps[:, wb, :],
                                             lhsT=preT_sb[:, wb, :],
                                             rhs=w_sb[:], start=False,
                                             stop=(wb == nb - 1))
                        h_sb = io.tile([128, nb, 128], F16, tag="hsb")
                        nc.scalar.activation(out=h_sb[:], in_=h_ps[:],
                                             func=ACT.Relu)
                        if pool:
                            for wb in range(nb):
                                nc.tensor.matmul(
                                    out=q_ps[:], lhsT=m_t[bi][:, wb, :],
                                    rhs=h_sb[:, wb, :],
                                    start=(b0 + wb == 0),
                                    stop=(b0 + wb == cfg.NW - 1))
                        else:
                            nc.sync.dma_start(out=out_r[:, b0:b1, :],
                                              in_=h_sb[:])
            if pool:
                q_sb = io.tile([cfg.G, cfg.HID], F32, tag="qsb")
                nc.vector.tensor_copy(out=q_sb[:], in_=q_ps[:])
                nc.sync.dma_start(out=out_d[:], in_=q_sb[:])
    nc.compile()
    return nc


RUNNER = None      # test harness hook: replaces the device-run path


def _run(nc, in_maps):
    if RUNNER is not None:
        return RUNNER(nc, in_maps)
    return run_bass_kernel_spmd(nc, in_maps, core_ids=list(range(NCORES))).results


def _win_major(a, cfg):
    """[NPC, D] row table -> [128, NW, D] (partition = slot in window)."""
    d = a.shape[1]
    out = np.zeros((cfg.NPAD, d), a.dtype)
    out[:a.shape[0]] = a
    return np.ascontiguousarray(
        out.reshape(cfg.NW, 128, d).transpose(1, 0, 2))


def _layer_inputs(cfg, plan, table_f16, bal, ident, iota, ones1, W, b):
    """table_f16: full [N, HID] fp16 node table (node-id order); bal is
    the balance_maps tuple -- the uploaded gather table is permuted to
    table row order and own rows follow node_at."""
    c_of, w_of, slot_of, qa, srow, node_at = bal
    inv = np.empty(cfg.N, np.int64)
    inv[srow] = np.arange(cfg.N)
    tab_perm = np.ascontiguousarray(table_f16[inv])
    maps = []
    for c in range(NCORES):
        flat = node_at[c].reshape(-1)
        mask = flat >= 0
        ownp = np.zeros((cfg.NPAD, cfg.HID), np.float16)
        ownp[mask] = table_f16[flat[mask]]
        # [128 feat, NW*128 nodes], window-major columns
        hTw = np.ascontiguousarray(
            ownp.reshape(cfg.NW, 128, cfg.HID).transpose(2, 0, 1)
            .reshape(cfg.HID, cfg.NW * 128))
        m = {"ht": tab_perm, "hTw": hTw, "ident": ident, "iota": iota,
             "ones1": ones1, "W": W,
             "brow": np.tile(b.reshape(1, -1), (1, WB)),
             "eidx": plan.idx_wrapped(c),
             "eslot": plan.col_arr(plan.slot, c, plan.NT),
             "ewgt": plan.col_arr(plan.wgt, c, plan.NOPS)}
        maps.append(m)
    return maps


NCS = {}


def gin_forward(cfg, x, edge_index, edge_weight, batch,
                W1, b1, W2, b2, W3, b3):
    src = np.asarray(edge_index[0], np.int64)
    dst = np.asarray(edge_index[1], np.int64)
    ew = np.asarray(edge_weight, np.float32)
    batch64 = np.asarray(batch, np.int64)
    bal = balance_maps(cfg, src, dst)
    c_of, w_of, slot_of, qa, srow, node_at = bal
    plan = Plan(cfg, src, dst, ew, c_of, w_of, slot_of, qa, srow)
    if "A" not in NCS:
        NCS["A"] = build_layer(cfg, plan, False)
        NCS["B"] = build_layer(cfg, plan, True)

    ident = np.eye(128, dtype=np.float16)
    iota = _iota_tile(128, 128)
    ones1 = np.ones((1, 128), np.float16)

    # M = (I + A^T) P  [N, G]: pool matrix, and per-graph node counts
    G = cfg.G
    M = np.bincount(src * G + batch64[dst], weights=ew.astype(np.float64),
                    minlength=cfg.N * G).reshape(cfg.N, G).astype(np.float32)
    M[np.arange(cfg.N), batch64] += 1.0
    counts = np.bincount(batch64, minlength=G).astype(np.float32)

    # Launch A: h1 = relu((x + A@x) @ W1 + b1)
    xt = np.ascontiguousarray(np.asarray(x, np.float32).astype(np.float16))
    maps = _layer_inputs(cfg, plan, xt, bal, ident, iota, ones1,
                         np.asarray(W1, np.float32).astype(np.float16),
                         np.asarray(b1, np.float32).astype(np.float16))
    res = _run(NCS["A"], maps)
    # h_out rows are in (core, window, slot) order; map back to node order
    h1_all = np.concatenate([res[c]["h_out"] for c in range(NCORES)])
    gidx = c_of * cfg.NPAD + w_of * 128 + slot_of
    h1 = np.ascontiguousarray(h1_all[gidx])

    # Launch B: h2 = relu((h1 + A@h1) @ W2 + b2); q_c = M_c^T @ h2_c
    maps = _layer_inputs(cfg, plan, h1, bal, ident, iota, ones1,
                         np.asarray(W2, np.float32).astype(np.float16),
                         np.asarray(b2, np.float32).astype(np.float16))
    M16 = M.astype(np.float16)
    for c, m in enumerate(maps):
        flat = node_at[c].reshape(-1)
        mask = flat >= 0
        mwin = np.zeros((cfg.NPAD, G), np.float16)
        mwin[mask] = M16[flat[mask]]
        m["M"] = np.ascontiguousarray(
            _win_major(mwin, cfg).reshape(128, cfg.NW * G))
    res = _run(NCS["B"], maps)
    q = np.zeros((G, cfg.HID), np.float32)
    for c in range(NCORES):
        q += res[c]["q"]

    out = q @ np.asarray(W3, np.float32) + \
        counts[:, None] * np.asarray(b3, np.float32)[None, :]
    return out.astype(np.float32)


def kernel(x, edge_index, edge_weight, batch, W1, b1, W2, b2, W3, b3):
    cfg = Cfg(N=100000, E=1600000)
    return gin_forward(cfg, x, edge_index, edge_weight, batch,
                       W1, b1, W2, b2, W3, b3)
